# revision 1
# baseline (speedup 1.0000x reference)
"""Trainium2 Bass kernel for nn_BondHead2 (dense_mlp), v2.

Per batch element b (8, one per NeuronCore):
    h = LN(gelu(x @ W1 + b1)); h = LN(gelu(h @ W2 + b2)); h = LN(gelu(h @ W3 + b3))
    out = h @ Wo + bo;  out = (out + out^T_{seq axes}) / 2

Design (per core, same math as v1 but restructured for engine/DMA balance):
  - tokens reordered block-major on the host: stats batch b == 128x128
    seq-block b, so each symmetrization pair streams out as soon as its two
    blocks finish layer 2 (sym overlaps the main loop's drain phase).
  - x prefetched one step ahead in 8-tile-chunk DMAs (18 total vs 288).
  - per-pair fused ops: one gelu / one square / one h-mul per 1024 tokens;
    h-mul in place (the g tile becomes the h tile).
  - 2-step software pipeline: the rstd DMA round-trip (PSUM stats -> fat
    reshape -> quake rsqrt -> DRAM -> stride-0 broadcast) gets most of a
    step of unrelated work before its h-multiply consumer; stat matmuls lag
    one pair behind the gelu/square chain so PE never waits on DVE.
  - rsqrt: quake seed on DVE (bit ops), 1 Newton step on the idle Pool
    engine (plain tensor-tensor ops only - hw GPSIMD has no STT/shift-32).
  - final projection stacked v-major in PSUM rows 0:80 (8-tile groups),
    one drain copy + 2 DMAs per group (36 vs 144 DMAs).
  - fp16 everywhere off-PSUM incl. the symmetrization and out_vm; 0.5 of
    the symmetrization and the centering/ln_g folds live in host weights.
"""

import numpy as np

import concourse.bacc as bacc
import concourse.bass as bass
import concourse.mybir as mybir
import concourse.tile as tile
from concourse.bass_utils import run_bass_kernel_spmd

F16 = mybir.dt.float16
F32 = mybir.dt.float32
U32 = mybir.dt.uint32
AF = mybir.ActivationFunctionType
OP = mybir.AluOpType

H = 64            # hidden dim
S = 384           # seq
T = S * S         # tokens per core (147456)
V = 5             # vocab
N = 512           # tokens per group-tile (free dim)
NT = T // (2 * N)  # 144 tiles (each tile = 2 groups x N tokens)
SB = 16           # tiles per stats batch
NB = NT // SB     # 9 batches (block-major: batch b == 128x128 block b)
NP = SB // 2      # 8 pairs per batch
BOFF = SB * N     # batch-local offset of group B (8192)
BLK = 2 * BOFF    # tokens per batch/block (16384)
EPS = 1e-5
MAGIC = 0x5F3759DF

_CACHE: dict = {}


def _build_nc():
    nc = bacc.Bacc("TRN2", target_bir_lowering=False, debug=False)

    # ---- external inputs ----
    xf = nc.dram_tensor("xf", (H, T), F16, kind="ExternalInput").ap()
    wmain = nc.dram_tensor("wmain", (128, 3 * 128), F16, kind="ExternalInput").ap()
    wstat = nc.dram_tensor("wstat", (128, SB * 32), F16, kind="ExternalInput").ap()
    wout = nc.dram_tensor("wout", (128, 8 * 80), F16, kind="ExternalInput").ap()
    biases = nc.dram_tensor("biases", (128, 3), F32, kind="ExternalInput").ap()
    id128 = nc.dram_tensor("id128", (128, 128), F16, kind="ExternalInput").ap()

    # ---- internal DRAM ----
    mobuf = nc.dram_tensor("mobuf", (V * T,), F16)          # [v, tok] linear
    r3buf = nc.dram_tensor("r3buf", (T,), F16)              # [i, j] linear
    out_vm = nc.dram_tensor("out_vm", (V * T,), F16, kind="ExternalOutput").ap()

    with tile.TileContext(nc) as tc:
        with tc.tile_pool(name="wpool", bufs=1) as wpool:
            from contextlib import ExitStack
            mstack = ExitStack()
            xpool = mstack.enter_context(tc.tile_pool(name="xpool", bufs=3))
            gpool = mstack.enter_context(tc.tile_pool(name="gpool", bufs=48))
            spool = mstack.enter_context(tc.tile_pool(name="spool", bufs=3))
            fpool = mstack.enter_context(tc.tile_pool(name="fpool", bufs=2))
            rpool = mstack.enter_context(tc.tile_pool(name="rpool", bufs=3))
            drpool = mstack.enter_context(
                tc.tile_pool(name="drpool", bufs=3, space="DRAM"))
            mps = mstack.enter_context(
                tc.tile_pool(name="mps", bufs=2, space="PSUM"))
            stps = mstack.enter_context(
                tc.tile_pool(name="stps", bufs=2, space="PSUM"))
            mops = mstack.enter_context(
                tc.tile_pool(name="mops", bufs=1, space="PSUM"))
            syps = mstack.enter_context(
                tc.tile_pool(name="syps", bufs=1, space="PSUM"))
            sy = mstack.enter_context(tc.tile_pool(name="sypool", bufs=3))

            # resident weights (wmain first - the first mains need only
            # wm + the first x chunk, so keep the DMA queue short up front)
            wm = wpool.tile([128, 3 * 128], F16)
            nc.sync.dma_start(out=wm[:], in_=wmain)
            bcol = wpool.tile([128, 3], F32)
            nc.sync.dma_start(out=bcol[:], in_=biases)
            ws = wpool.tile([128, SB * 32], F16)
            wo = wpool.tile([128, 8 * 80], F16)
            magic = wpool.tile([128, 1], U32)
            nc.vector.memset(magic[:], MAGIC)
            oneu = wpool.tile([128, 1], U32)
            nc.vector.memset(oneu[:], 1)
            chalf = wpool.tile([128, 1], F32)
            nc.vector.memset(chalf[:], -0.5)
            c15 = wpool.tile([128, 1], F32)
            nc.vector.memset(c15[:], 1.5)
            ceps = wpool.tile([128, 1], F32)
            nc.vector.memset(ceps[:], EPS)

            gstate = {}   # b -> list of NP pair tiles (g, then h in place)
            xstate = {}   # b -> list of chunk tiles
            mobank = {}   # current proj PSUM bank
            pending = {}  # (b, layer) -> (gnew, repl) awaiting h-mul

            def rsqrt_to(v_f32, out_ap):
                """out <- rsqrt(v) via quake seed + 1 Newton step, on Pool.

                Runs on the (otherwise idle) GPSIMD engine so the rstd chain
                never queues behind the DVE's bulk elementwise work.
                """
                sh = v_f32.shape
                y32 = fpool.tile(list(sh), F32, tag="nry")
                yi = y32[:].bitcast(U32)
                vi = v_f32.bitcast(U32)
                # bit-trick seed on DVE (Pool shift ops need 64-bit out on hw)
                nc.vector.tensor_tensor(
                    yi, vi, oneu[:].to_broadcast(sh), OP.logical_shift_right)
                nc.vector.tensor_tensor(
                    yi, magic[:].to_broadcast(sh), yi, OP.subtract)
                # Newton on Pool, plain TT ops only (hw GPSIMD limitation)
                t = fpool.tile(list(sh), F32, tag="nrt")
                nc.gpsimd.tensor_mul(t[:], y32[:], y32[:])
                nc.gpsimd.tensor_mul(t[:], t[:], v_f32)
                nc.gpsimd.tensor_mul(t[:], t[:], chalf[:].to_broadcast(sh))
                nc.gpsimd.tensor_tensor(
                    t[:], t[:], c15[:].to_broadcast(sh), OP.add)
                nc.gpsimd.tensor_mul(out_ap, t[:], y32[:])

            def prefetch_x(b):
                # batch 0 loads in 2-tile pieces so the first matmul isn't
                # gated on a 2.9us transfer; later batches prefetch a step
                # ahead so 8-tile chunks are fine.
                tpc = 2 if b == 0 else 8
                xch = []
                for c in range(SB // tpc):
                    xt = xpool.tile([128, tpc * N], F16, tag="x",
                                    name=f"xt{tpc}")
                    src = bass.AP(
                        tensor=xf.tensor,
                        offset=b * BLK + c * tpc * N,
                        ap=[[BOFF, 2], [T, 64], [1, tpc * N]],
                    )
                    nc.sync.dma_start(out=xt[:], in_=src)
                    xch.append(xt)
                xstate[b] = (xch, tpc)

            def do_layer(b, layer):
                gcur = gstate.get(b)
                statbank = stps.tile([64, N], F32, tag="stat")
                gnew = [None] * NP
                spair = [None] * NP

                def stats_for(p):
                    g = gnew[p]
                    s = spair[p]
                    for k in range(2):
                        bt = 2 * p + k
                        nc.tensor.matmul(
                            statbank[0:32], ws[:, 32 * bt:32 * bt + 32],
                            g[:, k * N:(k + 1) * N],
                            start=(bt == 0), stop=(bt == SB - 1),
                            skip_group_check=True,
                        )
                        nc.tensor.matmul(
                            statbank[32:64], ws[:, 32 * bt:32 * bt + 32],
                            s[:, k * N:(k + 1) * N],
                            start=(bt == 0), stop=(bt == SB - 1),
                            skip_group_check=True,
                        )
                        if layer == 2:
                            t8 = bt % 8
                            if t8 == 0:
                                mobank["cur"] = mops.tile(
                                    [128, N], F32, tag="mo", name="mob")
                            mob = mobank["cur"]
                            nc.tensor.matmul(
                                mob[0:80], wo[:, 80 * t8:80 * t8 + 80],
                                g[:, k * N:(k + 1) * N],
                                start=(t8 == 0), stop=(t8 == 7),
                                skip_group_check=True,
                                tile_position=(0, 0),
                            )
                            if t8 == 7:
                                mocp = spool.tile([80, N], F16, tag="mocp")
                                nc.scalar.copy(mocp[:], mob[0:80])
                                for g2 in range(2):
                                    dst = bass.AP(
                                        tensor=mobuf,
                                        offset=(b * BLK + g2 * BOFF
                                                + (bt - 7) * N),
                                        ap=[[T, V], [1, 8 * N]],
                                    )
                                    nc.sync.dma_start(
                                        out=dst, in_=mocp[40 * g2:40 * (g2 + 1)])

                # stats matmuls lag one pair behind the main/act/square chain
                # so the PE never waits on the same pair's DVE square.
                for p in range(NP + 2):
                    if p < NP:
                        mpair = mps.tile([128, 2, N], F32, tag="m")
                        for k in range(2):
                            bt = 2 * p + k
                            if layer == 0:
                                xch, tpc = xstate[b]
                                ch = xch[bt // tpc]
                                rhs = ch[:, (bt % tpc) * N:
                                         (bt % tpc + 1) * N]
                            else:
                                rhs = gcur[p][:, k * N:(k + 1) * N]
                            nc.tensor.matmul(
                                mpair[:, k, :],
                                wm[:, 128 * layer:128 * (layer + 1)],
                                rhs, start=True, stop=True,
                            )
                        g = gpool.tile([128, 2 * N], F16, tag="g")
                        nc.scalar.activation(
                            g[:], mpair[:].rearrange("p a n -> p (a n)"),
                            AF.Gelu, bias=bcol[:, layer:layer + 1], scale=1.0,
                        )
                        s = spool.tile([128, 2 * N], F16, tag="s",
                                       bufs=5)
                        nc.vector.tensor_mul(s[:], g[:], g[:])
                        gnew[p] = g
                        spair[p] = s
                    if p >= 2:
                        stats_for(p - 2)
                if layer == 0:
                    xstate.pop(b, None)
                # ---- batch-layer stats -> rstd ----
                # (drain on Act - GPSIMD cannot read PSUM, DVE is the pacer)
                rowboth = fpool.tile([64, N], F32, tag="row")
                nc.scalar.copy(rowboth[:], statbank[:])
                meanfat = fpool.tile([128, 128], F32, tag="meanfat")
                msqfat = fpool.tile([128, 128], F32, tag="msqfat")
                nc.sync.dma_start(
                    out=meanfat[:],
                    in_=rowboth[0:32, :].rearrange("p (q c) -> p q c", q=4),
                )
                nc.sync.dma_start(
                    out=msqfat[:],
                    in_=rowboth[32:64, :].rearrange("p (q c) -> p q c", q=4),
                )
                # var = (msq + eps) - mean^2   (in msqfat)
                sqf = fpool.tile([128, 128], F32, tag="sqf")
                nc.gpsimd.tensor_mul(sqf[:], meanfat[:], meanfat[:])
                nc.gpsimd.tensor_tensor(
                    msqfat[:], msqfat[:], sqf[:], OP.subtract)
                nc.gpsimd.tensor_tensor(
                    msqfat[:], msqfat[:], ceps[:].to_broadcast((128, 128)),
                    OP.add)
                if layer < 2:
                    rf16 = fpool.tile([128, 128], F16, tag="rf16")
                    rsqrt_to(msqfat[:], rf16[:])
                    rd = drpool.tile([2 * SB * N], F16, tag="rdram")
                    nc.sync.dma_start(
                        out=rd[:].rearrange("(a n) -> a n", a=128),
                        in_=rf16[:],
                    )
                    repl = rpool.tile([128, SB * N], F16, tag="repl")
                    rd_ap = rd[:]
                    for hf in range(8):
                        src2 = bass.AP(
                            tensor=rd_ap.tensor,
                            offset=rd_ap.offset + hf * (SB * N // 8),
                            ap=[[SB * N, 2], [0, 64], [1, SB * N // 8]],
                        )
                        nc.sync.dma_start(
                            out=repl[:, hf * (SB * N // 8):
                                     (hf + 1) * (SB * N // 8)],
                            in_=src2)
                    pending[(b, layer)] = (gnew, repl)
                else:
                    gstate.pop(b, None)
                    rf3 = fpool.tile([128, 128], F16, tag="rf16")
                    rsqrt_to(msqfat[:], rf3[:])
                    dst = bass.AP(
                        tensor=r3buf,
                        offset=b * BLK,
                        ap=[[BOFF, 2], [N, SB], [128, 4], [1, 128]],
                    )
                    nc.sync.dma_start(out=dst, in_=rf3[:])

            def apply_h(b, layer):
                gnew, repl = pending.pop((b, layer))
                for p in range(NP):
                    g = gnew[p]
                    nc.vector.tensor_mul(
                        g[:], g[:], repl[:, 2 * p * N:2 * (p + 1) * N])
                gstate[b] = gnew

            # ---- symmetrization (fp16; batch b == 128x128 block b) ----
            idt = wpool.tile([128, 128], F16)

            sym_pmap = {}

            def sym_prep(blk):
                mo = sy.tile([128, V, 128], F16, tag="mo_in", bufs=2)
                src = bass.AP(
                    tensor=mobuf, offset=blk * BLK,
                    ap=[[128, 128], [T, V], [1, 128]],
                )
                nc.sync.dma_start(out=mo[:], in_=src)
                r = sy.tile([128, 128], F16, tag="r_in", bufs=2)
                rsrc = bass.AP(
                    tensor=r3buf, offset=blk * BLK,
                    ap=[[128, 128], [1, 128]],
                )
                nc.sync.dma_start(out=r[:], in_=rsrc)
                p_ = sy.tile([128, V, 128], F16, tag="p", bufs=5)
                rb = bass.AP(tensor=r.tensor, offset=r.offset,
                             ap=[r.ap[0], [0, V], r.ap[1]])
                nc.vector.tensor_mul(p_[:], mo[:], rb)
                sym_pmap[blk] = p_

            def sym_transposes(p_):
                pt = syps.tile([128, V, 128], F16, tag="pt")
                for v in range(V):
                    nc.tensor.transpose(pt[:, v, :], p_[:, v, :], idt[:])
                return pt

            def sym_emit(pa, pt, bi, bj):
                o = sy.tile([128, V, 128], F16, tag="o", bufs=2)
                nc.vector.tensor_add(
                    o[:].rearrange("p a n -> p (a n)"),
                    pa[:].rearrange("p a n -> p (a n)"),
                    pt[:].rearrange("p a n -> p (a n)"),
                )
                d1 = bass.AP(
                    tensor=out_vm.tensor, offset=bi * 128 * S + bj * 128,
                    ap=[[S, 128], [T, V], [1, 128]],
                )
                nc.sync.dma_start(out=d1, in_=o[:])

            def sym_pair(bi, bj):
                pa = sym_pmap.pop(3 * bi + bj)
                if bi == bj:
                    sym_emit(pa, sym_transposes(pa), bi, bj)
                else:
                    pb = sym_pmap.pop(3 * bj + bi)
                    sym_emit(pa, sym_transposes(pb), bi, bj)
                    sym_emit(pb, sym_transposes(pa), bj, bi)

            # software pipeline (2-step skew): each rstd DMA round-trip gets
            # at least ~2/3 of a step of unrelated work before its h-multiply
            # consumer.
            # sym pair {bi,bj} is ready once blocks 3bi+bj and 3bj+bi have
            # finished layer 2 (step b+3); interleave pairs into the loop so
            # sym work fills pipeline-drain bubbles instead of trailing.
            # Defer sym work into the pipeline-drain steps (NB..NB+2) where
            # PE/DVE/DMA would otherwise idle; a pair still can't run before
            # its gate (both blocks through layer 2).
            sym_sched = {}
            sym_prep_sched = {}
            for bi in range(3):
                for bj in range(bi + 1):
                    gate = max(3 * bi + bj, 3 * bj + bi) + 3
                    sym_sched.setdefault(gate, []).append((bi, bj))
                    for blk in {3 * bi + bj, 3 * bj + bi}:
                        sym_prep_sched.setdefault(gate, []).append(blk)

            prefetch_x(0)
            nc.sync.dma_start(out=ws[:], in_=wstat)
            nc.sync.dma_start(out=wo[:], in_=wout)
            nc.sync.dma_start(out=idt[:], in_=id128)
            for step in range(NB + 3):
                if 0 <= step - 2 < NB:
                    apply_h(step - 2, 0)
                    do_layer(step - 2, 1)
                if 0 <= step - 3 < NB:
                    apply_h(step - 3, 1)
                    do_layer(step - 3, 2)
                if step < NB:
                    do_layer(step, 0)
                if step + 1 < NB:
                    prefetch_x(step + 1)
                for blk in sym_prep_sched.get(step, []):
                    sym_prep(blk)
                for (bi, bj) in sym_sched.get(step, []):
                    sym_pair(bi, bj)
            mstack.close()

    nc.compile()
    return nc


def _prep_weights(inputs):
    W1 = np.asarray(inputs["W1"], np.float64)
    W2 = np.asarray(inputs["W2"], np.float64)
    W3 = np.asarray(inputs["W3"], np.float64)
    Wo = np.asarray(inputs["Wo"], np.float64)
    b1 = np.asarray(inputs["b1"], np.float64)
    b2 = np.asarray(inputs["b2"], np.float64)
    b3 = np.asarray(inputs["b3"], np.float64)
    bo = np.asarray(inputs["bo"], np.float64)
    ln_g = np.asarray(inputs["ln_g"], np.float64)
    ln_b = np.asarray(inputs["ln_b"], np.float64)

    C = np.eye(H) - np.ones((H, H)) / H
    F = C @ np.diag(ln_g)
    Ws = [W1, F @ W2, F @ W3]
    bs = [b1, b2 + W2.T @ ln_b, b3 + W3.T @ ln_b]
    Woh = 0.5 * (F @ Wo)
    boh = (bo + Wo.T @ ln_b).astype(np.float32)

    wmain = np.zeros((128, 3 * 128), np.float16)
    for l, W in enumerate(Ws):
        wmain[0:64, 128 * l:128 * l + 64] = W.astype(np.float16)
        wmain[64:128, 128 * l + 64:128 * l + 128] = W.astype(np.float16)
    # stats lhsT variant bt: rows 0-15 of the PSUM bank hold group-A means
    # (row bt), rows 16-31 group-B means (row SB+bt)
    wstat = np.zeros((128, SB * 32), np.float16)
    for bt in range(SB):
        wstat[0:64, 32 * bt + bt] = np.float16(1 / 64)
        wstat[64:128, 32 * bt + SB + bt] = np.float16(1 / 64)
    # v-major projection: variant t8 sends (grp, v) to PSUM row
    # 40*grp + 8*v + t8
    wout = np.zeros((128, 8 * 80), np.float16)
    w16 = Woh.astype(np.float16)
    for t8 in range(8):
        for v in range(V):
            wout[0:64, 80 * t8 + 8 * v + t8] = w16[:, v]
            wout[64:128, 80 * t8 + 40 + 8 * v + t8] = w16[:, v]
    biases = np.zeros((128, 3), np.float32)
    for l, bb in enumerate(bs):
        biases[0:64, l] = bb.astype(np.float32)
        biases[64:128, l] = bb.astype(np.float32)
    id128 = np.eye(128, dtype=np.float16)
    return dict(wmain=wmain, wstat=wstat, wout=wout, biases=biases,
                id128=id128), boh


def _prep_x(xb):
    """[S, S, H] fp32 -> [H, T] fp16 in block-major token order."""
    t = xb.reshape(3, 128, 3, 128, H).transpose(0, 2, 1, 3, 4).reshape(T, H)
    return np.ascontiguousarray(t.T).astype(np.float16)


def kernel(**inputs):
    if "nc" not in _CACHE:
        _CACHE["nc"] = _build_nc()
    nc = _CACHE["nc"]
    weights, boh = _prep_weights(inputs)

    x = np.asarray(inputs["x"])  # [8, S, S, H] fp32
    in_maps = []
    for b in range(8):
        m = dict(weights)
        m["xf"] = _prep_x(x[b])
        in_maps.append(m)

    res = run_bass_kernel_spmd(nc, in_maps, core_ids=list(range(8)))
    outs = []
    for b in range(8):
        vm = res.results[b]["out_vm"].reshape(V, S, S).astype(np.float32)
        outs.append(vm.transpose(1, 2, 0) + boh[None, None, :])
    return np.stack(outs).astype(np.float32)

